# revision 24
# baseline (speedup 1.0000x reference)
"""Per-subject linear dispatch (MoE routing) — expert-grouped ragged GEMM.

Reference:
    h   = x @ W[subject_ids] + b[subject_ids]          # [B, S, D]
    h   = h * (1 - mask) + mask_token * mask
    out = concat([subj_table[subject_ids][:, None, :], h], axis=1)

Masked rows (mask==1) are exactly mask_token -> host fills them. Only the
~50% unmasked rows hit the device. Rows are grouped BY EXPERT across the
whole batch and sharded over 8 cores, so each core DMAs only the 2-3
expert weight matrices it needs (vs 4 gathered copies in a per-sample
layout).

Device formulation (per core): W is the STATIONARY matmul operand and the
packed x rows are the MOVING operand, so the row count needs no 128-row
tile quantization — PSUM groups are [128 d-chunk, Rb<=512 rows]:

    psum[dd, rows] = sum_kc W_e[kc*128:+128, dd*128:+128].T @ xT[kc, rows]
    out_sb = fp16(psum + b_e[dd*128:+128])     (per-partition bias on copyback)

SPMD constraint: all 8 cores run one program, so the slot layout
(NSEG expert slots x R_s rows) is a common compile-time profile chosen by
a packer from the actual per-expert row counts; cores pad unused slot
capacity with junk rows (discarded on host). fp16 in/out (gate is 2e-2;
fp16 lands ~4e-4), fp32 PSUM accumulation.
"""

import math
import os
import time as _time
from collections import defaultdict

import numpy as np

import concourse.bass as bass
import concourse.mybir as mybir
from concourse import bacc
from concourse.bass_utils import run_bass_kernel_spmd

B, S, C, D = 32, 512, 512, 1024
NCORES = 8
P = 128
NKC = C // P           # 4 contraction chunks
NKD = D // P           # 8 output-partition chunks
RBMAX = 512            # PSUM bank free dim (fp32)

# measured on HW: 340 B/ns aggregate DMA, ~2.3 moving rows/ns on PE (fp16)
DMA_BPNS = 340.0
PE_ROWS_PNS = 2.3

TRACE = False
LAST_EXEC_NS = None
LAST_RESULTS = None

_nc_cache = {}


# ---------------------------------------------------------------- packer
#
# SPMD: every core pays the COMMON slot profile (R_0 >= ... >= R_{n-1}) in
# PE rows, x/out DMA (sum R_s) and W DMA (nseg x 1MB) -- regardless of its
# actual data. Each rank has exactly 8 slot instances (one per core), so an
# assignment exists iff expert row counts can be cut into pieces fitting
# the 8xnseg slot pool: a pure counting problem, checked by memoized DFS.

def _cost(prof):
    # exec ~= max(PE work, input-feed) + fixed epilogue; PE = 32 passes/row
    # at ~2.38 rows/ns; the input feed (x + W) must stay ahead of the PE.
    rt, nseg = sum(prof), len(prof)
    pe = (32.0 / PE_ROWS_PNS) * rt + 2000.0
    feed = (rt * C * 2 + nseg * C * D * 2) / DMA_BPNS + 1000.0
    return max(pe, feed)


def _feasible(sizes, prof, node_budget=4000):
    """sizes desc; prof desc. Returns per-expert piece-count vectors
    [(k_0..k_{nseg-1}), ...] or None."""
    nseg = len(prof)
    # cheap necessary condition: piece count bound at the largest rank
    if sum(-(-r // prof[0]) for r in sizes) > NCORES * nseg:
        return None

    def vectors(r, rem):
        # minimal piece-count vectors covering r within remaining slots
        out = []

        def rec(i, ks, cov):
            if cov >= r:
                out.append(tuple(ks + [0] * (nseg - len(ks))))
                return
            if i == nseg:
                return
            hi = min(rem[i], -(-(r - cov) // prof[i]) if prof[i] else 0)
            for k in range(hi, -1, -1):
                rec(i + 1, ks + [k], cov + k * prof[i])

        rec(0, [], 0)
        # drop dominated vectors
        out2 = [v for v in out
                if not any(w != v and all(wi <= vi for wi, vi in zip(w, v))
                           for w in out)]
        return out2

    memo = {}
    nodes = [0]

    def dfs(i, rem):
        if i == len(sizes):
            return []
        nodes[0] += 1
        if nodes[0] > node_budget:
            return None
        key = (i, rem)
        if key in memo:
            return memo[key]
        # capacity prune: remaining slot capacity must cover remaining rows
        if sum(a * b for a, b in zip(rem, prof)) < sum(sizes[i:]):
            memo[key] = None
            return None
        res = None
        for v in vectors(sizes[i], rem):
            nrem = tuple(a - b for a, b in zip(rem, v))
            if min(nrem) < 0:
                continue
            sub = dfs(i + 1, nrem)
            if sub is not None:
                res = [v] + sub
                break
        memo[key] = res
        return res

    return dfs(0, tuple([NCORES] * nseg))


def pack_experts(expert_rows):
    """expert_rows: dict e -> n_rows (>0). Returns (profile, assignment) where
    assignment[e] = [(core, slot, n), ...] covering n_rows in order."""
    order = sorted(expert_rows, key=lambda e: -expert_rows[e])
    sizes = [expert_rows[e] for e in order]
    total = sum(sizes)
    t0 = max(1, math.ceil(total / NCORES))
    rmax = max(sizes)

    def rgrid(step):
        return range(step, ((rmax + step - 1) // step + 1) * step, step)

    cands = []
    for nseg in (1, 2, 3, 4):
        if nseg * C * D * 2 / DMA_BPNS > _cost([t0]):
            continue
        if nseg == 1:
            cands.extend([(rmax + pad,) for pad in (0, 16, 64)])
        elif nseg == 2:
            cands.extend((r0, r1) for r0 in rgrid(16) for r1 in rgrid(16)
                         if r1 <= r0 and t0 <= r0 + r1 <= 2.2 * t0)
        elif nseg == 3:
            cands.extend((r0, r1, r2)
                         for r0 in rgrid(32) for r1 in rgrid(32)
                         for r2 in rgrid(16)
                         if r2 <= r1 <= r0 and t0 <= r0 + r1 + r2 <= 1.6 * t0)
        else:
            cands.extend((r0, r1, r2, r3)
                         for r0 in rgrid(32) for r1 in rgrid(32)
                         for r2 in rgrid(32) for r3 in rgrid(32)
                         if r3 <= r2 <= r1 <= r0
                         and t0 <= r0 + r1 + r2 + r3 <= 1.35 * t0)
    cands.sort(key=_cost)
    prof, kvecs = None, None
    deadline = _time.monotonic() + 20.0
    for cand in cands[:20000]:
        kv = _feasible(sizes, cand)
        if kv is not None:
            prof, kvecs = cand, kv
            break
        if _time.monotonic() > deadline:
            break
    if prof is None:  # fallback: rmax-wide ranks, one expert per slot
        prof = tuple([rmax] * math.ceil(len(sizes) / NCORES))
        kvecs = _feasible(sizes, prof, node_budget=10**6)
        assert kvecs is not None

    # materialize pieces -> (core, slot): rank i slots assigned round-robin
    next_core = [0] * len(prof)
    assignment = defaultdict(list)
    for ei, kv in enumerate(kvecs):
        e, r = order[ei], sizes[ei]
        for rank, k in enumerate(kv):
            for _ in range(k):
                take = min(r, prof[rank])
                c = next_core[rank]
                next_core[rank] += 1
                assignment[e].append((c, rank, take))
                r -= take
        assert r <= 0, (e, r, kv)

    # reorder ranks smallest-first: slot 0 computes first, so a small slot 0
    # minimizes the bytes needed before the PE can start.
    perm = sorted(range(len(prof)), key=lambda i: prof[i])
    rank_map = {old: new for new, old in enumerate(perm)}
    prof2 = tuple(prof[i] for i in perm)
    assignment2 = {
        e: [(c, rank_map[s], n) for c, s, n in pl]
        for e, pl in assignment.items()
    }
    return prof2, assignment2


def _blocks_of(prof):
    """[(slot, r0_global, rb), ...] with rb<=RBMAX; final block split for a
    short output-DMA tail."""
    blocks = []
    off = 0
    for s, rs in enumerate(prof):
        nb = -(-rs // RBMAX)
        lo, extra = divmod(rs, nb)
        r = 0
        for i in range(nb):
            rb = lo + (1 if i < extra else 0)
            blocks.append([s, off + r, rb])
            r += rb
        off += rs
    if blocks and blocks[-1][2] > 192:
        s, r0, rb = blocks.pop()
        h = rb // 2
        blocks.append([s, r0, h])
        blocks.append([s, r0 + h, rb - h])
    return [tuple(b) for b in blocks]


# ---------------------------------------------------------------- builder

def _build(prof):
    nseg = len(prof)
    rt = sum(prof)
    blocks = _blocks_of(prof)
    ngrp = len(blocks) * NKD
    rb0 = blocks[0][2]               # first block's rows = xt segment A
    nwarm = int(os.environ.get("BASS_WARMUP", "8"))
    fp16 = mybir.dt.float16
    fp32 = mybir.dt.float32

    nc = bacc.Bacc(
        "TRN2",
        target_bir_lowering=False,
        debug=False,
        num_devices=NCORES,
    )
    rbb = rt - rb0                   # segment-B rows (blocks 1..)
    # xt split into two tensors so each DMA moves per-partition-contiguous
    # runs (slicing rows out of one tensor would fragment the runs)
    xta_d = nc.dram_tensor("xta", [P, NKC, rb0], fp16, kind="ExternalInput").ap()
    xtb_d = nc.dram_tensor("xtb", [P, NKC, max(rbb, 1)], fp16,
                           kind="ExternalInput").ap()
    # W layout [P, NKD, NKC, 128]: slot 0 streams in dd-pair slices (2KB
    # runs) so the PE can start during the clock ramp; later slots go as
    # whole-slot DMAs (8KB runs) on the GPSIMD ring, gated behind segment A
    # so they never compete with the first feed.
    w_d = nc.dram_tensor("w", [nseg, P, NKD, NKC, P], fp16, kind="ExternalInput").ap()
    bias_d = nc.dram_tensor("bias", [P, nseg * NKD], fp32, kind="ExternalInput").ap()
    out_d = nc.dram_tensor("out", [P, NKD * rt], fp16, kind="ExternalOutput").ap()

    xta = nc.alloc_sbuf_tensor("xta_sb", [P, NKC, rb0], fp16).ap()
    xtb = nc.alloc_sbuf_tensor("xtb_sb", [P, NKC, max(rbb, 1)], fp16).ap()
    w = [nc.alloc_sbuf_tensor(f"w_sb{s}", [P, NKD, NKC, P], fp16).ap()
         for s in range(nseg)]
    bias = nc.alloc_sbuf_tensor("bias_sb", [P, nseg * NKD], fp32).ap()
    ot = [
        nc.alloc_sbuf_tensor(f"ot{bi}", [P, NKD, rb], fp16).ap()
        for bi, (_, _, rb) in enumerate(blocks)
    ]
    scratch = nc.alloc_sbuf_tensor("scratch", [P, RBMAX], fp16).ap()
    ps = [nc.alloc_psum_tensor(f"ps{k}", [P, RBMAX], fp32).ap() for k in range(8)]

    xta_sem = nc.alloc_semaphore("xta_sem")
    xtb_sem = nc.alloc_semaphore("xtb_sem")
    w0_sem = [nc.alloc_semaphore(f"w0p{p}") for p in range(4)]
    ws_sem = [nc.alloc_semaphore(f"ws{s}") for s in range(1, nseg)]
    bias_sem = nc.alloc_semaphore("bias_sem")
    mm_done = nc.alloc_semaphore("mm_done")
    cp_act = nc.alloc_semaphore("cp_act")
    cp_dve = nc.alloc_semaphore("cp_dve")
    out_sem = nc.alloc_semaphore("out_sem")
    scratch_sem = nc.alloc_semaphore("scratch_sem")

    # groups: bank g%8, copyback engine = DVE for even g, ACT for odd g
    groups = [(bi, b[0], b[1], b[2], dd) for bi, b in enumerate(blocks)
              for dd in range(NKD)]

    def cp_count(g):
        # counter value on g's engine after g's copyback completes
        return g // 2 + 1

    with nc.Block() as block:

        @block.sync
        def _(sp):
            sp.dma_start(bias[:], bias_d).then_inc(bias_sem, 16)
            sp.dma_start(w[0][:, 0:2], w_d[0, :, 0:2]).then_inc(w0_sem[0], 16)
            sp.dma_start(xta[:], xta_d).then_inc(xta_sem, 16)
            for p in range(1, 4):
                sp.dma_start(
                    w[0][:, 2 * p:2 * p + 2], w_d[0, :, 2 * p:2 * p + 2]
                ).then_inc(w0_sem[p], 16)
            if rbb > 0:
                sp.dma_start(xtb[:], xtb_d).then_inc(xtb_sem, 16)
            for bi, (s, r0, rb) in enumerate(blocks):
                # all 8 dd-groups of block bi copied back?
                last_g = bi * NKD + NKD - 1
                sp.wait_ge(cp_dve, (last_g - 1) // 2 + 1)
                sp.wait_ge(cp_act, last_g // 2 + 1)
                sp.dma_start(
                    out_d[:, NKD * r0:NKD * (r0 + rb)], ot[bi][:]
                ).then_inc(out_sem, 16)

        @block.gpsimd
        def _(gps):
            gps.memset(scratch[:], 0.0).then_inc(scratch_sem, 1)

        @block.tensor
        def _(pe):
            seen = set()

            def need(sem, val):
                if (sem, val) not in seen:
                    pe.wait_ge(sem, val)
                    seen.add((sem, val))

            # HAM warm-up: PE clock ramps while the first DMAs land
            pe.wait_ge(scratch_sem, 1)
            for _ in range(nwarm):
                pe.matmul(ps[7][:], scratch[:, 0:P], scratch[:],
                          start=True, stop=True)

            for g, (bi, s, r0, rb, dd) in enumerate(groups):
                if g >= 8:
                    pg = g - 8
                    sem = cp_dve if pg % 2 == 0 else cp_act
                    pe.wait_ge(sem, cp_count(pg))
                if s == 0:
                    need(w0_sem[dd // 2], 16)
                else:
                    need(ws_sem[s - 1], 16)
                need(xta_sem if bi == 0 else xtb_sem, 16)
                rhs = xta if bi == 0 else xtb
                roff = r0 if bi == 0 else r0 - rb0
                for kc in range(NKC):
                    mm = pe.matmul(
                        ps[g % 8][:, 0:rb],
                        w[s][:, dd, kc, :],
                        rhs[:, kc, roff:roff + rb],
                        start=(kc == 0),
                        stop=(kc == NKC - 1),
                    )
                    if kc == NKC - 1:
                        mm.then_inc(mm_done, 1)

        @block.scalar
        def _(act):
            # later W slots ride ACT's HWDGE ring (qScalarDynamicHW), in
            # parallel with SP's, gated behind segment A so they don't
            # compete with the first feed
            if nseg > 1:
                act.wait_ge(xta_sem, 16)
                for s in range(1, nseg):
                    act.dma_start(w[s][:], w_d[s]).then_inc(ws_sem[s - 1], 16)
            first = True
            for g, (bi, s, r0, rb, dd) in enumerate(groups):
                if g % 2 != 1:
                    continue
                if first:
                    act.wait_ge(bias_sem, 16)
                    first = False
                act.wait_ge(mm_done, g + 1)
                act.activation(
                    ot[bi][:, dd, 0:rb],
                    ps[g % 8][:, 0:rb],
                    mybir.ActivationFunctionType.Identity,
                    bias=bias[:, s * NKD + dd:s * NKD + dd + 1],
                    scale=1.0,
                ).then_inc(cp_act, 1)

        @block.vector
        def _(dve):
            first = True
            for g, (bi, s, r0, rb, dd) in enumerate(groups):
                if g % 2 != 0:
                    continue
                if first:
                    dve.wait_ge(bias_sem, 16)
                    first = False
                dve.wait_ge(mm_done, g + 1)
                dve.tensor_scalar_add(
                    ot[bi][:, dd, 0:rb],
                    ps[g % 8][:, 0:rb],
                    bias[:, s * NKD + dd:s * NKD + dd + 1],
                ).then_inc(cp_dve, 1)

    nc.compile()
    return nc


def get_nc(prof):
    if prof not in _nc_cache:
        _nc_cache[prof] = _build(prof)
    return _nc_cache[prof]


# ---------------------------------------------------------------- host

def prepare_inputs(x, one_m, W, b, sid):
    """Returns (prof, in_maps, decode) where decode[c] =
    [(block_cols, rows_b, rows_t, nvalid, rb), ...]."""
    expert_rows = {}
    rows_of = {}
    for e in range(W.shape[0]):
        bs = np.nonzero(sid == e)[0]
        if len(bs) == 0:
            continue
        rb_, rt_ = [], []
        for bb in bs:
            ts = np.nonzero(one_m[bb] > 0.5)[0]
            rb_.append(np.full(len(ts), bb, dtype=np.int64))
            rt_.append(ts)
        rows_b = np.concatenate(rb_) if rb_ else np.empty(0, np.int64)
        rows_t = np.concatenate(rt_) if rt_ else np.empty(0, np.int64)
        if len(rows_b) == 0:
            continue
        expert_rows[e] = len(rows_b)
        rows_of[e] = (rows_b, rows_t)

    prof, assignment = pack_experts(expert_rows)
    nseg, rt = len(prof), sum(prof)
    offs = np.cumsum([0] + list(prof))
    blocks = _blocks_of(prof)

    rb0 = _blocks_of(prof)[0][2]
    xt = np.zeros((NCORES, P, NKC, rt), np.float16)
    w = np.zeros((NCORES, nseg, P, NKD, NKC, P), np.float16)
    bias = np.zeros((NCORES, P, nseg * NKD), np.float32)
    # per (core, slot): (expert, rows_b, rows_t)
    slot_data = [[None] * nseg for _ in range(NCORES)]
    for e, pl in assignment.items():
        rows_b, rows_t = rows_of[e]
        pos = 0
        wt = (W[e].astype(np.float16)
              .reshape(NKC, P, NKD, P).transpose(1, 2, 0, 3))
        bt = b[e].astype(np.float32).reshape(NKD, P).T
        for c, s, n in pl:
            rb_, rt_ = rows_b[pos:pos + n], rows_t[pos:pos + n]
            pos += n
            slot_data[c][s] = (e, rb_, rt_)
            xg = x[rb_, rt_, :].astype(np.float16)          # [n, C]
            xt[c, :, :, offs[s]:offs[s] + n] = (
                xg.T.reshape(NKC, P, n).transpose(1, 0, 2)
            )
            w[c, s] = wt
            bias[c, :, s * NKD:(s + 1) * NKD] = bt

    xta = np.ascontiguousarray(xt[:, :, :, :rb0])
    xtb = (np.ascontiguousarray(xt[:, :, :, rb0:]) if rt > rb0
           else np.zeros((NCORES, P, NKC, 1), np.float16))
    in_maps = [
        {"xta": xta[c], "xtb": xtb[c], "w": w[c], "bias": bias[c]}
        for c in range(NCORES)
    ]

    decode = []
    for c in range(NCORES):
        dec = []
        for s, r0, rb in blocks:
            sd = slot_data[c][s]
            if sd is None:
                continue
            _, rows_b, rows_t = sd
            lo = r0 - offs[s]                 # block-local start within slot
            nvalid = min(len(rows_b) - lo, rb)
            if nvalid <= 0:
                continue
            dec.append((NKD * r0, rows_b[lo:lo + nvalid],
                        rows_t[lo:lo + nvalid], nvalid, rb))
        decode.append(dec)
    return prof, in_maps, decode


def kernel(x, mask, W, b, subj_table, mask_token, subject_ids):
    global LAST_EXEC_NS, LAST_RESULTS
    x = np.asarray(x, dtype=np.float32)
    mask = np.asarray(mask, dtype=np.float32)
    W = np.asarray(W, dtype=np.float32)
    b = np.asarray(b, dtype=np.float32)
    subj_table = np.asarray(subj_table, dtype=np.float32)
    mask_token = np.asarray(mask_token, dtype=np.float32)
    sid = np.asarray(subject_ids).astype(np.int64)

    m = mask[:, :, 0]
    one_m = np.float32(1.0) - m

    out = np.empty((B, S + 1, D), dtype=np.float32)
    out[:, 0, :] = subj_table[sid]
    out[:, 1:, :] = mask_token[0]        # masked rows are exactly mask_token

    if not (one_m > 0.5).any():
        return out

    prof, in_maps, decode = prepare_inputs(x, one_m, W, b, sid)
    nc = get_nc(prof)
    res = run_bass_kernel_spmd(nc, in_maps, list(range(NCORES)), trace=TRACE)
    LAST_EXEC_NS = res.exec_time_ns
    LAST_RESULTS = res

    for c in range(NCORES):
        dev = res.results[c]["out"]                     # [P, NKD*rt] fp16
        for col0, rows_b, rows_t, nvalid, rb in decode[c]:
            chunk = dev[:, col0:col0 + NKD * rb].reshape(P, NKD, rb)
            vecs = chunk.transpose(2, 1, 0).reshape(rb, D)[:nvalid]
            out[rows_b, 1 + rows_t, :] = vecs.astype(np.float32)
    return out
